# revision 35
# baseline (speedup 1.0000x reference)
"""CLS-AttentionPool2d Trainium2 kernel (8 NeuronCores, data-parallel over batch).

Math refactoring (single CLS query => tiny attention):
  tokens[j] = x[b,:,j]                         (j = 0..1023, native [C, HW] layout)
  mean      = tokens.mean(j);  cls = mean + pos0
  q  = (Wq @ cls + bq) / sqrt(C)
  qblk[k, (s,h)] = q_s[k] * [head(k) == h]     (block-diag arrangement)
  m  = Wk.T @ qblk                             # m[c, slot] per-head key-projected query
  scores[slot, j]    = m.T @ x_tokens  (+ KP-term for pos_emb, KP = Wk @ pos.T host-precomputed)
  scores[slot, cls]  = rowmean(token scores) + KP-cls-term     (mean is linear)
  p = softmax(scores)  ;  p' = p_tok + p_cls/1024  (folds CLS-mean into token weights)
  w  = p'.T @ (tokens via PE/xbar transpose) + p'.T @ pos_tok + p_cls * pos0adj
  out = Wv @ w + bv                            (per-head block of Wv)

The q.bk term is constant across j => dropped (softmax shift invariance).

Scheduling notes: the Tile scheduler orders instructions by its internal
cost model, which under-estimates DMA contention, so tc.tile_wait_until
staggers the x-load issue and pins group-1 work (means, transposes) behind
group-0's critical chain. The x loads are SWDGE DMAs (the only path that
casts f32->bf16) split into j-halves to keep the SWDGE descriptor ring
pipelined. x transposes are split between the PE (idle early / between
phases) and the DMA xbar; the output projection runs once, batched over
all 8 batches.
"""

import math
import numpy as np

import concourse.bass as bass
import concourse.mybir as mybir
import concourse.tile as tile
from concourse import bacc
from concourse.bass import ts
from concourse.bass_utils import run_bass_kernel_spmd

F32 = mybir.dt.float32
BF16 = mybir.dt.bfloat16
AX = mybir.AxisListType
ALU = mybir.AluOpType
ACTF = mybir.ActivationFunctionType

B, C, HW = 64, 512, 1024
NH, DH = 8, 64
NCORES = 8
BPC = B // NCORES          # 8 batches per core
GRP = 4                    # batches per group (2 groups per core)
NGRP = BPC // GRP
CT = C // 128              # 4 c-chunks
JT = HW // 128             # 8 j-chunks
ISQ = 1.0 / math.sqrt(C)

# per-(group, slot) transpose route: True => DMA xbar, False => PE
# (each xbar costs ~4.6us of all-16-SDMA time and competes with the x
# stream, so only two slots go through it, early, when DMA has headroom)
XBAR = {(0, 0): False, (0, 1): False, (0, 2): False, (0, 3): False,
        (1, 0): False, (1, 1): False, (1, 2): False, (1, 3): True}

_CACHE = {}


def _build_nc():
    nc = bacc.Bacc("TRN2", target_bir_lowering=False, debug=False,
                   num_devices=NCORES)

    # ---- DRAM I/O ----
    xs = nc.dram_tensor("xs", [BPC, C, HW], F32, kind="ExternalInput")
    wqt = nc.dram_tensor("wqt", [128, CT, C], BF16, kind="ExternalInput")
    wk = nc.dram_tensor("wk", [128, CT, C], BF16, kind="ExternalInput")
    wvt = nc.dram_tensor("wvt", [128, CT, C], BF16, kind="ExternalInput")
    kp = nc.dram_tensor("kp", [128, CT, HW + 1], BF16, kind="ExternalInput")
    postok = nc.dram_tensor("postok", [128, JT, C], BF16, kind="ExternalInput")
    pos0row = nc.dram_tensor("pos0row", [1, C], BF16, kind="ExternalInput")
    pos0 = nc.dram_tensor("pos0", [128, CT], F32, kind="ExternalInput")
    bqs = nc.dram_tensor("bqs", [128, CT], F32, kind="ExternalInput")
    bv = nc.dram_tensor("bv", [128, CT], F32, kind="ExternalInput")
    mask32 = nc.dram_tensor("mask32", [128, CT, 32], F32, kind="ExternalInput")
    ident = nc.dram_tensor("ident", [128, 128], BF16, kind="ExternalInput")
    identf = nc.dram_tensor("identf", [128, 128], F32, kind="ExternalInput")
    out_d = nc.dram_tensor("out", [BPC, C], F32, kind="ExternalOutput")

    with tile.TileContext(nc) as tc:
        with (
            tc.tile_pool(name="persist", bufs=1) as pp,
            tc.tile_pool(name="big", bufs=1) as bigp,
            tc.tile_pool(name="work", bufs=2) as wp,
            tc.tile_pool(name="psA", bufs=1, space="PSUM") as psA,
            tc.tile_pool(name="psB", bufs=3, space="PSUM") as psB,
            tc.tile_pool(name="psC", bufs=1, space="PSUM") as psC,
            tc.tile_pool(name="psD", bufs=1, space="PSUM") as psD,
        ):
            # ---- persistent tiles ----
            wqt_s = pp.tile([128, CT, C], BF16)
            wk_s = pp.tile([128, CT, C], BF16)
            wvt_s = pp.tile([128, CT, C], BF16)
            kp_s = pp.tile([128, CT, HW + 1], BF16)
            postok_s = pp.tile([128, JT, C], BF16)
            pos0row_s = pp.tile([1, C], BF16)
            pos0_s = pp.tile([128, CT], F32)
            bqs_s = pp.tile([128, CT], F32)
            bv_s = pp.tile([128, CT], F32)
            mask_s = pp.tile([128, CT, 32], F32)
            ident_s = pp.tile([128, 128], BF16)
            identf_s = pp.tile([128, 128], F32)
            sums = pp.tile([128, CT, BPC], F32)
            junk = pp.tile([128, HW], BF16)
            wt_all = pp.tile([128, CT, NGRP, 128], BF16)

            throttle = pp.tile([128, 4], BF16)  # noqa: F841 (keeps SBUF layout
            # identical to the hardware-verified build)

            # x loads must go through gpsimd (only SWDGE DMAs can cast
            # f32->bf16). The SWDGE ring only sustains a few outstanding
            # transfers, so each load is split into two j-halves to keep the
            # ring pipelined, and batches 4-7 get staggered issue times so
            # they don't round-robin-steal bandwidth from the first group.
            xwaits = [0.0, 0.003, 0.006, 0.009, 0.022, 0.0275, 0.033, 0.0385]
            xb = []
            for i in range(BPC):
                xt = bigp.tile([128, CT, HW], BF16, tag="xb", bufs=8)
                xb.append(xt)
                src = xs[i].rearrange("(t p) j -> p t j", p=128)
                with tc.tile_wait_until(xwaits[i], enable=xwaits[i] > 0):
                    for h in range(2):
                        nc.gpsimd.dma_start(
                            out=xt[:, :, 512 * h:512 * (h + 1)],
                            in_=src[:, :, 512 * h:512 * (h + 1)])

            def emit_mean(i, eng):
                if eng == "a":
                    for t in range(CT):
                        nc.scalar.activation(
                            junk[:], xb[i][:, t, :], ACTF.Copy,
                            accum_out=sums[:, t, i:i + 1])
                elif eng == "v":
                    nc.vector.tensor_reduce(
                        sums[:, :, i:i + 1].rearrange("p t one -> p (t one)"),
                        xb[i][:], axis=AX.X, op=ALU.add)
                else:  # split: chunks 0-1 on ACT, 2-3 on DVE (halves latency)
                    for t in range(2):
                        nc.scalar.activation(
                            junk[:], xb[i][:, t, :], ACTF.Copy,
                            accum_out=sums[:, t, i:i + 1])
                    nc.vector.tensor_reduce(
                        sums[:, 2:4, i:i + 1].rearrange("p t one -> p (t one)"),
                        xb[i][:, 2:4, :], axis=AX.X, op=ALU.add)

            # tables: early wave (q/m/scores), late wave (w/output)
            for dst, src in [(wqt_s, wqt), (wk_s, wk), (kp_s, kp),
                             (mask_s, mask32), (bqs_s, bqs), (pos0_s, pos0),
                             (ident_s, ident)]:
                nc.sync.dma_start(out=dst[:], in_=src[:])

            toks = [None] * BPC

            def emit_transpose(g, s):
                src = xb[g * GRP + s]
                tok = bigp.tile([128, CT, JT, 128], BF16, tag="tok", bufs=8)
                toks[g * GRP + s] = tok
                if XBAR[(g, s)]:
                    nc.sync.dma_start_transpose(
                        tok[:].rearrange("p t j c -> p (t j) c"), src[:])
                else:
                    for jp in range(JT // 2):
                        tp2 = psB.tile([128, CT, 2, 128], BF16, tag="psB")
                        for jj in range(2):
                            for t in range(CT):
                                nc.tensor.transpose(
                                    tp2[:, t, jj, :],
                                    src[:, t, ts(2 * jp + jj, 128)],
                                    ident_s[:])
                        if jp % 2 == 1:
                            nc.scalar.copy(tok[:, :, 2 * jp:2 * jp + 2, :],
                                           tp2[:])
                        else:
                            nc.vector.tensor_copy(
                                tok[:, :, 2 * jp:2 * jp + 2, :], tp2[:])

            def phase_qm(g):
                cls_all = wp.tile([128, CT, GRP], BF16, tag="cls")
                nc.vector.scalar_tensor_tensor(
                    out=cls_all[:], in0=sums[:, :, g * GRP:(g + 1) * GRP],
                    scalar=1.0 / HW,
                    in1=pos0_s[:, :, None].broadcast_to([128, CT, GRP]),
                    op0=ALU.mult, op1=ALU.add)

                q_ps = psC.tile([128, CT, GRP], F32, tag="psC")
                for mc in range(CT):
                    for tk in range(CT):
                        nc.tensor.matmul(
                            q_ps[:, mc, :], wqt_s[:, tk, ts(mc, 128)],
                            cls_all[:, tk, :],
                            start=(tk == 0), stop=(tk == CT - 1))
                q_sb = wp.tile([128, CT, GRP], F32, tag="qsb")
                nc.vector.scalar_tensor_tensor(
                    out=q_sb[:], in0=q_ps[:], scalar=ISQ,
                    in1=bqs_s[:, :, None].broadcast_to([128, CT, GRP]),
                    op0=ALU.mult, op1=ALU.add)

                qblk = wp.tile([128, CT, GRP, 32], BF16, tag="qblk")
                nc.vector.tensor_mul(
                    qblk[:],
                    q_sb[:, :, :, None].broadcast_to([128, CT, GRP, 32]),
                    mask_s[:, :, None, :].broadcast_to([128, CT, GRP, 32]))
                qblk_f = qblk[:].rearrange("p t s u -> p t (s u)")

                m_ps = psC.tile([128, CT, 128], F32, tag="psC")
                for mc in range(CT):
                    for tk in range(CT):
                        nc.tensor.matmul(
                            m_ps[:, mc, :], wk_s[:, tk, ts(mc, 128)],
                            qblk_f[:, tk, :],
                            start=(tk == 0), stop=(tk == CT - 1))
                m_sb = wp.tile([128, CT, 128], BF16, tag="msb")
                nc.vector.tensor_copy(m_sb[:], m_ps[:])
                return qblk_f, m_sb

            def phase_scores(g, qblk_f, m_sb):
                sc_ps = psA.tile([128, 3, 512], F32, tag="psA")
                for bk2, (j0, wdt) in enumerate(((0, 512), (512, 512),
                                                 (1024, 1))):
                    for tk in range(CT):
                        nc.tensor.matmul(
                            sc_ps[:, bk2, 0:wdt], qblk_f[:, tk, :],
                            kp_s[:, tk, j0:j0 + wdt],
                            start=(tk == 0), stop=False,
                            skip_group_check=True)
                for s in range(GRP):
                    for tk in range(CT):
                        for jc in range(2):
                            nc.tensor.matmul(
                                sc_ps[32 * s:32 * s + 8, jc, :],
                                m_sb[:, tk, 32 * s:32 * s + 8],
                                xb[g * GRP + s][:, tk, ts(jc, 512)],
                                start=False, stop=False,
                                tile_position=(0, 32 * s),
                                skip_group_check=True)
                redcol = wp.tile([128, 1], F32, tag="redcol")
                nc.vector.reduce_sum(redcol[:], sc_ps[:, 0:2, :], axis=AX.XY)
                nc.vector.scalar_tensor_tensor(
                    out=sc_ps[:, 2, 0:1], in0=redcol[:], scalar=1.0 / HW,
                    in1=sc_ps[:, 2, 0:1], op0=ALU.mult, op1=ALU.add)
                return sc_ps

            def phase_softmax(g, sc_ps):
                p_sb = wp.tile([128, HW + 1], BF16, tag="psb")
                sumexp = wp.tile([128, 1], F32, tag="sumexp")
                se2 = wp.tile([128, 1], F32, tag="se2")
                nc.scalar.activation(p_sb[:, 0:HW], sc_ps[:, 0:2, :],
                                     ACTF.Exp, scale=1.0, accum_out=sumexp[:])
                nc.scalar.activation(p_sb[:, HW:HW + 1], sc_ps[:, 2, 0:1],
                                     ACTF.Exp, scale=1.0, accum_out=se2[:])
                nc.vector.tensor_add(sumexp[:], sumexp[:], se2[:])
                rz = wp.tile([128, 1], F32, tag="rz")
                nc.vector.reciprocal(rz[:], sumexp[:])
                pcls_sc = wp.tile([128, 1], F32, tag="pclssc")
                nc.vector.tensor_scalar_mul(pcls_sc[:], p_sb[:, HW:HW + 1],
                                            1.0 / HW)
                nc.vector.tensor_scalar_add(p_sb[:, 0:HW], p_sb[:, 0:HW],
                                            pcls_sc[:])
                return p_sb, rz

            def phase_pT(g, p_sb):
                pT = wp.tile([128, JT, 128], BF16, tag="pT")
                for half in range(2):
                    tp = psB.tile([128, 4, 128], BF16, tag="psB")
                    for k in range(4):
                        jc = half * 4 + k
                        nc.tensor.transpose(tp[:, k, :], p_sb[:, ts(jc, 128)],
                                            ident_s[:])
                    nc.vector.tensor_copy(
                        pT[:, half * 4:(half + 1) * 4, :], tp[:])
                pTc_ps = psB.tile([1, 128], BF16, tag="psB")
                nc.tensor.transpose(pTc_ps[:], p_sb[:, HW:HW + 1], ident_s[:])
                pTc = wp.tile([1, 128], BF16, tag="pTc")
                nc.vector.tensor_copy(pTc[:], pTc_ps[:])
                return pT, pTc

            def phase_w(g, pT, pTc, rz):
                w_ps = psD.tile([128, C], F32, tag="psD")
                for s in range(GRP):
                    for jc in range(JT):
                        nc.tensor.matmul(
                            w_ps[32 * s:32 * s + 32, :],
                            pT[:, jc, 32 * s:32 * s + 32],
                            toks[g * GRP + s][:, :, jc, :],
                            start=(jc == 0), stop=(jc == JT - 1),
                            tile_position=(0, 32 * s))
                for jc in range(JT):
                    nc.tensor.matmul(w_ps[:], pT[:, jc, :], postok_s[:, jc, :],
                                     start=False, stop=False,
                                     skip_group_check=True)
                nc.tensor.matmul(w_ps[:], pTc[:], pos0row_s[:],
                                 start=False, stop=True,
                                 skip_group_check=True)
                w_sb = wp.tile([128, C], BF16, tag="wsb")
                nc.vector.tensor_scalar_mul(w_sb[:], w_ps[:], rz[:])

                tp3 = psB.tile([128, CT, 128], BF16, tag="psB")
                for mc in range(CT):
                    nc.tensor.transpose(tp3[:, mc, :],
                                        w_sb[:, ts(mc, 128)], ident_s[:])
                nc.vector.tensor_copy(wt_all[:, :, g, :], tp3[:])

            # ================== schedule ==================
            # group 0: per-batch transpose + mean, interleaved so each
            # in-order engine queue consumes in data-arrival order
            emit_transpose(0, 0)
            emit_mean(0, "v")
            emit_transpose(0, 1)
            emit_mean(1, "a")
            emit_transpose(0, 2)
            emit_mean(2, "a")
            emit_transpose(0, 3)
            emit_mean(3, "s")
            # late tables: after group-0's xbar waits in the sync FIFO (so
            # they don't steal bandwidth from the critical x0-3 stream) but
            # before group-1's, arriving in time for phase_w(0)
            for dst, src in [(postok_s, postok), (wvt_s, wvt),
                             (pos0row_s, pos0row), (bv_s, bv),
                             (identf_s, identf)]:
                nc.sync.dma_start(out=dst[:], in_=src[:])
            qblk0, msb0 = phase_qm(0)
            sc0 = phase_scores(0, qblk0, msb0)
            p0, rz0 = phase_softmax(0, sc0)
            # group-1 means split across ACT+DVE (half-batch each), emitted
            # behind exp0 / group-0's softmax DVE ops so they never delay
            # the group-0 critical path; group-1's PE transposes go after
            # w0 (filling the wait for mean7), its xbar transposes run
            # post-stream when the SDMA engines are free
            with tc.tile_wait_until(0.030):
                emit_mean(4, "s")
            with tc.tile_wait_until(0.036):
                emit_mean(5, "s")
            pT0, pTc0 = phase_pT(0, p0)
            phase_w(0, pT0, pTc0, rz0)
            with tc.tile_wait_until(0.041):
                emit_mean(6, "s")
            with tc.tile_wait_until(0.047):
                emit_mean(7, "s")
            with tc.tile_wait_until(0.041):
                emit_transpose(1, 2)
            with tc.tile_wait_until(0.047):
                emit_transpose(1, 3)
            with tc.tile_wait_until(0.043):
                emit_transpose(1, 0)
                emit_transpose(1, 1)
            # group 1
            qblk1, msb1 = phase_qm(1)
            sc1 = phase_scores(1, qblk1, msb1)
            p1, rz1 = phase_softmax(1, sc1)
            pT1, pTc1 = phase_pT(1, p1)
            phase_w(1, pT1, pTc1, rz1)

            # ---------------- output projection (all 8 batches) ----------
            out_ps = psC.tile([128, CT, NGRP, GRP], F32, tag="psC")
            for h in range(NH):
                pr, hi = h // 2, 64 * (h % 2)
                for tk in range(CT):
                    nc.tensor.matmul(
                        out_ps[hi:hi + 64, pr, :, :],
                        wvt_s[:, tk, h * DH:(h + 1) * DH],
                        wt_all[:, tk, :, h::32],
                        start=(tk == 0), stop=(tk == CT - 1),
                        tile_position=(0, hi),
                        skip_group_check=True)
            out_sb = wp.tile([128, CT, NGRP, GRP], F32, tag="outsb")
            for pr in range(CT):
                nc.vector.tensor_scalar_add(out_sb[:, pr, :, :],
                                            out_ps[:, pr, :, :],
                                            bv_s[:, pr:pr + 1])
            # transpose to [batch, c] so all 8 rows store as one DMA of
            # contiguous 2KB lines (8 per-batch stores each scatter 512
            # 4-byte descriptors and serialize on the sync FIFO)
            oT_ps = psB.tile([8, CT, 128], F32, tag="psB")
            for mc in range(CT):
                nc.tensor.transpose(
                    oT_ps[:, mc, :],
                    out_sb[:, mc, :, :].rearrange("p g s -> p (g s)"),
                    identf_s[:])
            oT_sb = wp.tile([8, CT, 128], F32, tag="oTsb")
            nc.vector.tensor_copy(oT_sb[:], oT_ps[:])
            nc.sync.dma_start(
                out=out_d[:].rearrange("b (t c) -> b t c", t=CT),
                in_=oT_sb[:])

    nc.compile()
    return nc


def _prep(pos_emb, Wq, bq, Wk, bk, Wv, bv):
    import ml_dtypes
    bf = ml_dtypes.bfloat16

    def ptn(v):  # [512] -> [128, CT], c = t*128 + p
        return np.ascontiguousarray(v.reshape(CT, 128).T)

    def chunkk(w):  # [512, N] -> [128, CT, N], k = t*128 + p
        return np.ascontiguousarray(w.reshape(CT, 128, -1).transpose(1, 0, 2))

    order = np.r_[1:HW + 1, 0]
    kpm = Wk.astype(np.float64) @ pos_emb[order].astype(np.float64).T
    p1 = pos_emb[1:].sum(axis=0)
    pos0adj = pos_emb[0] - p1 / HW
    mask = np.zeros((128, CT, 32), np.float32)
    for p in range(128):
        for t in range(CT):
            h = (t * 128 + p) // DH
            mask[p, t, h] = 1.0

    postok_nat = pos_emb[1:].reshape(JT, 128, C).transpose(1, 0, 2)
    return {
        "wqt": chunkk(np.ascontiguousarray(Wq.T)).astype(bf),
        "wk": chunkk(Wk).astype(bf),
        "wvt": chunkk(np.ascontiguousarray(Wv.T)).astype(bf),
        "kp": chunkk(kpm.astype(np.float32)).astype(bf),
        "postok": np.ascontiguousarray(postok_nat).astype(bf),
        "pos0row": np.ascontiguousarray(pos0adj.reshape(1, C)).astype(bf),
        "pos0": ptn(pos_emb[0]),
        "bqs": ptn(bq * ISQ),
        "bv": np.ascontiguousarray(bv.reshape(CT, 128).T),
        "mask32": mask,
        "ident": np.eye(128, dtype=np.float32).astype(bf),
        "identf": np.eye(128, dtype=np.float32),
    }


def kernel(x, pos_emb, Wq, bq, Wk, bk, Wv, bv, num_heads):
    assert int(num_heads) == NH
    x = np.asarray(x, dtype=np.float32).reshape(B, C, HW)
    if "nc" not in _CACHE:
        _CACHE["nc"] = _build_nc()
    nc = _CACHE["nc"]
    shared = _prep(np.asarray(pos_emb, np.float32), np.asarray(Wq, np.float32),
                   np.asarray(bq, np.float32), np.asarray(Wk, np.float32),
                   np.asarray(bk, np.float32), np.asarray(Wv, np.float32),
                   np.asarray(bv, np.float32))
    in_maps = []
    for i in range(NCORES):
        m = dict(shared)
        m["xs"] = np.ascontiguousarray(x[i * BPC:(i + 1) * BPC])
        in_maps.append(m)
    res = run_bass_kernel_spmd(nc, in_maps, list(range(NCORES)))
    out = np.concatenate([res.results[i]["out"] for i in range(NCORES)], axis=0)
    return out.astype(np.float32)
